# revision 16
# baseline (speedup 1.0000x reference)
"""Multi-head attention TRN2 kernel.

Problem: x[2,2048,128] -> MHA with 8 heads of dim 128 (inner 1024) -> out[2,2048,128].
Sharding: 8 cores; core c handles batch b=c//4 and heads (2*(c%4), 2*(c%4)+1).
Each core returns the transposed partial output (its two heads' contribution to
y @ Wp); host sums the 4 cores of each batch, transposes, and adds the constant
row bv @ Wp + bp.

Math notes (exact rewrites, not approximations):
- head_dim == n_embd == 128, so the Q/K projections collapse into a single
  128x128 matrix per head: logits = (x M + c) x^T with M = scale*Wq Wk^T and
  c = scale*Wk bq (K bias drops out of softmax entirely; Q bias becomes c).
  The kernel never computes Q or K.
- Likewise V/output projections collapse: out^T = sum_h N_h^T (x^T P_h / r_h)
  with N_h = Wv_h Wp_h, so the kernel never computes V either; the AV matmul
  contracts raw x blocks against the exp tiles, and the V bias contributes
  exactly bv to y (softmax rows sum to 1), folding into the host-side
  constant row.
- Logits have |.| of only a few units, so exp() runs without max-subtraction.

Schedule (v2, exp-bound design):
- The ACT engine is the bottleneck (64 MB of exp per iteration at 1 elem/
  cycle/lane).  QK writes block PAIRS into one fixed [128,2048] PSUM tile and
  exp runs once per pair at N=2048, halving ACT instruction overhead.
- Fully rotated software pipeline: each attention loop defers its last 6 AV
  consumes and its softmax tail (rowsum broadcast, reciprocal, normalize) into
  the NEXT loop's first blocks.  The loop4->loop1 edge crosses the For_i
  boundary, so those deferred emissions reference only fixed-address tiles
  (pt2_fix pair tiles, s3_fix rowsum, ps_z, Zn, G) -- no live Python closures
  over pool tiles cross the edge.  All engines finish each body pass nearly
  simultaneously, so the For_i drain barrier costs little.
- Next-iteration inputs are prefetched inside loop4: each xT chunk DMA is
  emitted right after its last QK read, and the G = (xM+c)^T projection
  chunks follow their xT chunk, so body start has no lead-in work.
- PSUM (8 banks): att pair tile [128,2048] f32 (4) + Z accum [128,1024] f32
  (2) + a 2x[128,512] utility pool (2) time-shared by G projection, rowsum
  broadcast, and output-projection chunks.
- Rowsum: pair-halves add on DVE (s0), mid-tree s1 adds on the otherwise-idle
  Pool engine, tail s1[3]/s2/s3 on DVE; an all-ones stationary matmul
  broadcasts the partition-sums; reciprocal + multiply normalizes.
"""

import sys

sys.path.insert(0, "/opt/trn_rl_repo")

import contextlib
import math
from collections import defaultdict

import numpy as np

import concourse.bass as bass
import concourse.mybir as mybir
import concourse.tile as tile
from concourse import bacc
from concourse.bass_utils import run_bass_kernel_spmd

N_CORES = 8
B, L, F = 2, 2048, 128
NH = 8
HEADS_PER_CORE = 2
LH = 1024  # attention is processed in two l-halves of 1024 columns
F32 = mybir.dt.float32
BF16 = mybir.dt.bfloat16
n_blk = L // F  # 16 a-blocks of 128
n_pair = n_blk // 2  # 8 fused exp pairs per loop
FIRST_CONSUME = 6  # in-loop AV consume j runs at block j+6; j=10..15 deferred


UNROLL = 2  # iterations per For_i body (amortizes the drain barrier)


def build_nc(loop_n: int = 1, sim_init: bool = False):
    del sim_init  # prologue always initializes pipeline-carried state
    assert loop_n == 1 or loop_n % UNROLL == 0, loop_n
    nc = bacc.Bacc("TRN2", target_bir_lowering=False, debug=False, num_devices=N_CORES)
    xT_d = nc.dram_tensor("xT", [F, L], BF16, kind="ExternalInput").ap()
    xnt_d = nc.dram_tensor("xnt", [F, L], BF16, kind="ExternalInput").ap()
    mw_d = nc.dram_tensor("mw", [HEADS_PER_CORE, F, F], BF16, kind="ExternalInput").ap()
    nw_d = nc.dram_tensor("nw", [HEADS_PER_CORE, F, F], BF16, kind="ExternalInput").ap()
    cv_d = nc.dram_tensor("cv", [HEADS_PER_CORE, F, 1], F32, kind="ExternalInput").ap()
    outT_d = nc.dram_tensor("outT", [F, L], BF16, kind="ExternalOutput").ap()

    Exp = mybir.ActivationFunctionType.Exp

    with tile.TileContext(nc) as tc, nc.allow_low_precision(
        reason="bf16 tensors feed the PE at full rate; accumulation stays fp32"
    ):
        with (
            tc.tile_pool(name="fixed", bufs=1) as fixed,
            tc.tile_pool(name="ptp", bufs=8) as ptp,
            tc.tile_pool(name="s0p", bufs=6) as s0p,
            tc.tile_pool(name="scr", bufs=2) as scr,
            tc.tile_pool(name="attp", bufs=3, space="PSUM") as attp,
            tc.tile_pool(name="psZp", bufs=1, space="PSUM") as psZp,
        ):
            # ---- fixed-address tiles (live across For_i bodies) ----------
            mw_sb = [
                fixed.tile([F, F], BF16, tag=f"mw{h}", name=f"mw{h}")
                for h in range(HEADS_PER_CORE)
            ]
            nw_sb = [
                fixed.tile([F, F], BF16, tag=f"nw{h}", name=f"nw{h}")
                for h in range(HEADS_PER_CORE)
            ]
            cv_sb = [
                fixed.tile([F, 1], F32, tag=f"cv{h}", name=f"cv{h}")
                for h in range(HEADS_PER_CORE)
            ]
            xTc = [
                fixed.tile([F, 512], BF16, tag=f"xTc{j}", name=f"xTc{j}")
                for j in range(4)
            ]
            xntc = [
                fixed.tile([F, LH], BF16, tag=f"xntc{j}", name=f"xntc{j}")
                for j in range(2)
            ]
            G = [
                fixed.tile([F, L], BF16, tag=f"G{h}", name=f"G{h}")
                for h in range(HEADS_PER_CORE)
            ]
            Zn = [
                fixed.tile([F, L], BF16, tag=f"Zn{h}", name=f"Zn{h}")
                for h in range(HEADS_PER_CORE)
            ]
            # exp tiles for blocks 10..15 are fixed so the deferred AV
            # consumes can be authored without closures over pool tiles
            pt_fix = [
                fixed.tile([F, LH], BF16, tag=f"ptf{k}", name=f"ptf{k}")
                for k in range(6)
            ]
            s1_fixed = [
                fixed.tile([F, LH], BF16, tag=f"s1f{k}", name=f"s1f{k}")
                for k in range(4)
            ]
            s06_fix = fixed.tile([F, LH], BF16, tag="s06f", name="s06f")

            ps_z = psZp.tile([F, LH], F32, tag="zz", name="ps_z")

            def xT_blk(i):  # lhsT for att block i
                return xTc[i // 4][:, (i % 4) * F : (i % 4 + 1) * F]

            def xnt_blk(j):  # lhsT for AV block j
                return xntc[j // 8][:, (j % 8) * F : (j % 8 + 1) * F]

            def g_pair(q):
                """Two thunks writing G[0]/G[1] chunk q via one big psum
                tile's halves (alloc on the first)."""
                box = {}

                def th_h(h):
                    def th():
                        if h == 0:
                            box["t"] = attp.tile([F, LH], F32, tag="big", name=f"g{q}")
                        cs = slice(h * 512, (h + 1) * 512)
                        nc.tensor.matmul(box["t"][:, cs], lhsT=mw_sb[h][:], rhs=xTc[q][:])
                        nc.vector.tensor_scalar_add(
                            G[h][:, q * 512 : (q + 1) * 512], box["t"][:, cs], cv_sb[h][:]
                        )
                    return th

                return th_h(0), th_h(1)

            def g_single(h, q):
                """Standalone G chunk emission (own psum tile); used where
                the paired tile's closure cannot cross the For_i edge."""
                def th():
                    pg = attp.tile([F, LH], F32, tag="big", name=f"gs{h}{q}")
                    nc.tensor.matmul(pg[:, 0:512], lhsT=mw_sb[h][:], rhs=xTc[q][:])
                    nc.vector.tensor_scalar_add(
                        G[h][:, q * 512 : (q + 1) * 512], pg[:, 0:512], cv_sb[h][:]
                    )
                return th

            def out_pass(lhh):
                """alloc+mm(c0), mm(c1)+evac(c0), evac(c1) thunks for the
                output projection of l-half lhh (reads prev-iter Zn)."""
                box = {}
                lo = lhh * LH

                def mm(c):
                    def th():
                        if c == 0:
                            box["t"] = attp.tile([F, LH], F32, tag="big", name=f"o{lhh}")
                        for hh in range(HEADS_PER_CORE):
                            nc.tensor.matmul(
                                box["t"][:, c * 512 : (c + 1) * 512],
                                lhsT=nw_sb[hh][:],
                                rhs=Zn[hh][:, lo + c * 512 : lo + (c + 1) * 512],
                                start=hh == 0,
                                stop=hh == HEADS_PER_CORE - 1,
                            )
                    return th

                def evac(c):
                    def th():
                        ob = scr.tile([F, 512], BF16, tag="ob", name="ob", bufs=4)
                        nc.vector.tensor_copy(ob[:], box["t"][:, c * 512 : (c + 1) * 512])
                        nc.sync.dma_start(
                            out=outT_d[:, lo + c * 512 : lo + (c + 1) * 512], in_=ob[:]
                        )
                    return th

                return mm(0), mm(1), evac(0), evac(1)

            def edge_injects(hp, lp):
                """Deferred tail of the previous loop (head hp, l-half lp):
                6 AV consumes from the fixed exp tiles, rowsum broadcast,
                reciprocals, and the Zn normalize.  Only fixed-address tiles
                are referenced, so this works across the For_i edge."""
                lo = lp * LH
                box = {}

                def consume_prev(j):
                    def th():
                        pf = pt_fix[j - 10]
                        for c in range(2):
                            nc.tensor.matmul(
                                ps_z[:, c * 512 : (c + 1) * 512],
                                lhsT=xnt_blk(j),
                                rhs=pf[:, c * 512 : (c + 1) * 512],
                                start=False,
                                stop=j == n_blk - 1,
                            )
                    return th

                def bcast():
                    pb = attp.tile([F, LH], F32, tag="big", name="bsum")
                    box["b"] = pb
                    for k in range(4):
                        for c in range(2):
                            cs = slice(c * 512, (c + 1) * 512)
                            nc.tensor.matmul(
                                pb[:, cs],
                                lhsT=ones_mat[:],
                                rhs=s1_fixed[k][:, cs],
                                start=k == 0,
                                stop=k == 3,
                            )

                def recips():
                    for c in range(2):
                        cs = slice(c * 512, (c + 1) * 512)
                        rbc = scr.tile([F, 512], F32, tag="rbc", name="rbc")
                        nc.vector.reciprocal(rbc[:], box["b"][:, cs])
                        box[f"r{c}"] = rbc

                def muls():
                    for c in range(2):
                        nc.vector.tensor_mul(
                            Zn[hp][:, lo + c * 512 : lo + (c + 1) * 512],
                            ps_z[:, c * 512 : (c + 1) * 512],
                            box[f"r{c}"][:],
                        )

                def late_s0():
                    box["s0l"] = s0p.tile([F, LH], BF16, tag="s0", name="s0l")
                    nc.vector.tensor_add(box["s0l"][:], pt_fix[4][:], pt_fix[5][:])

                def late_s1():
                    nc.vector.tensor_add(s1_fixed[3][:], s06_fix[:], box["s0l"][:])

                inj = defaultdict(list)
                inj[0].append(late_s0)
                inj[1].append(late_s1)
                inj[(0, "pre")].append(consume_prev(10))
                inj[(1, "pre")].append(consume_prev(11))
                inj[(2, "pre")].append(consume_prev(12))
                inj[2].append(bcast)
                inj[(3, "pre")].append(consume_prev(13))
                inj[3].append(recips)
                inj[(4, "pre")].append(consume_prev(14))
                inj[(5, "pre")].append(consume_prev(15))
                inj[5].append(muls)
                return inj

            def att_loop(h, lh, inj, post=()):
                """One (head, l-half) attention loop over 16 a-blocks.

                Per block: injected thunks, then the in-loop AV consume
                (j = i-6), then the QK matmuls + exp + rowsum adds.  The
                last 6 consumes and the softmax tail are emitted by the
                NEXT loop via edge_injects."""
                lo = lh * LH
                pts = [None] * n_blk
                s0 = [None] * n_pair
                for i in range(n_blk):
                    for th in inj.get((i, "pre"), ()):
                        th()
                    j = i - FIRST_CONSUME
                    if 0 <= j:
                        for c in range(2):
                            nc.tensor.matmul(
                                ps_z[:, c * 512 : (c + 1) * 512],
                                lhsT=xnt_blk(j),
                                rhs=pts[j][:, c * 512 : (c + 1) * 512],
                                start=j == 0,
                                stop=False,
                            )
                    ps_att = attp.tile([F, LH], F32, tag="big", name="ps_att")
                    for c in range(2):
                        nc.tensor.matmul(
                            ps_att[:, c * 512 : (c + 1) * 512],
                            lhsT=xT_blk(i),
                            rhs=G[h][:, lo + c * 512 : lo + (c + 1) * 512],
                        )
                    for th in inj.get(i, ()):
                        th()
                    pt = (
                        pt_fix[i - 10]
                        if i >= 10
                        else ptp.tile([F, LH], BF16, tag="pt", name="pt")
                    )
                    pts[i] = pt
                    nc.scalar.activation(pt[:], ps_att[:], Exp)
                    # rowsum tree: s0 pair adds on DVE; s1 adds on the Pool
                    # engine into fixed tiles; the PE's accumulated broadcast
                    # (next loop) sums the four s1 levels, so no serial s2/s3
                    # chain trails the loop's final exp.
                    if i % 2 == 1 and i < 15:
                        p = i // 2
                        if p == 6:
                            s0[p] = s06_fix
                        else:
                            s0[p] = s0p.tile([F, LH], BF16, tag="s0", name="s0")
                        nc.vector.tensor_add(s0[p][:], pts[i - 1][:], pts[i][:])
                    if i in (5, 9, 13):
                        k = (i - 5) // 4
                        nc.gpsimd.tensor_add(
                            s1_fixed[k][:], s0[2 * k][:], s0[2 * k + 1][:]
                        )
                for th in post:
                    th()

            # --- one-time setup + prologue (outside the timed loop) --------
            ones_stage = fixed.tile([F, F], F32, tag="ones_stage", name="ones_stage")
            nc.vector.memset(ones_stage[:], 1.0)
            ones_mat = fixed.tile([F, F], BF16, tag="ones_mat", name="ones_mat")
            nc.vector.tensor_copy(ones_mat[:], ones_stage[:])
            warm = fixed.tile([F, 1], BF16, tag="warm", name="warm")
            nc.scalar.activation(warm[:], ones_stage[:, 0:1], Exp)
            # initialize pipeline-carried state read by the first body pass
            for h in range(HEADS_PER_CORE):
                nc.vector.memset(Zn[h][:], 0.0)
            for k in range(6):
                nc.vector.memset(pt_fix[k][:], 0.0)
            for k in range(4):
                nc.vector.memset(s1_fixed[k][:], 1.0)
            nc.vector.memset(s06_fix[:], 1.0)
            # open the ps_z accumulation group the first body pass's deferred
            # consumes (start=False) will continue
            for c in range(2):
                nc.tensor.matmul(
                    ps_z[:, c * 512 : (c + 1) * 512],
                    lhsT=ones_mat[:],
                    rhs=pt_fix[0][:, c * 512 : (c + 1) * 512],
                    start=True,
                    stop=False,
                )
            for h in range(HEADS_PER_CORE):
                nc.sync.dma_start(out=mw_sb[h][:], in_=mw_d[h])
            for j in range(4):
                nc.sync.dma_start(out=xTc[j][:], in_=xT_d[:, j * 512 : (j + 1) * 512])
            for h in range(HEADS_PER_CORE):
                nc.sync.dma_start(out=cv_sb[h][:], in_=cv_d[h])
            for h in range(HEADS_PER_CORE):
                nc.sync.dma_start(out=nw_sb[h][:], in_=nw_d[h])
            for j in range(2):
                nc.sync.dma_start(out=xntc[j][:], in_=xnt_d[:, j * LH : (j + 1) * LH])
            for q in range(4):
                a, b = g_pair(q)
                a()
                b()

            def dma_mw_cv():
                for h in range(HEADS_PER_CORE):
                    nc.sync.dma_start(out=mw_sb[h][:], in_=mw_d[h])
                for h in range(HEADS_PER_CORE):
                    nc.sync.dma_start(out=cv_sb[h][:], in_=cv_d[h])

            def dma_nw():
                for h in range(HEADS_PER_CORE):
                    nc.sync.dma_start(out=nw_sb[h][:], in_=nw_d[h])

            def dma_xnt():
                for j in range(2):
                    nc.sync.dma_start(
                        out=xntc[j][:], in_=xnt_d[:, j * LH : (j + 1) * LH]
                    )

            def dma_xT(c):
                def th():
                    nc.sync.dma_start(
                        out=xTc[c][:], in_=xT_d[:, c * 512 : (c + 1) * 512]
                    )
                return th

            loop_cm = (
                tc.For_i(
                    0,
                    loop_n // UNROLL,
                    1,
                    hint_engines=(
                        mybir.EngineType.PE,
                        mybir.EngineType.Activation,
                        mybir.EngineType.DVE,
                        mybir.EngineType.SP,
                        mybir.EngineType.Pool,
                    ),
                )
                if loop_n > 1
                else contextlib.nullcontext()
            )
            def emit_iter():
                # loop1: prev body's loop4 tail + leftover G chunks + dmas +
                # out-projection of the previous iteration's Zn
                g3a, g3b = g_pair(3)
                o1 = out_pass(0)
                o2 = out_pass(1)
                inj1 = edge_injects(1, 1)
                inj1[0].insert(0, g3a)
                inj1[0].insert(0, dma_xT(3))
                inj1[1].insert(0, g3b)
                inj1[2].insert(0, g_single(1, 2))
                inj1[2].append(dma_mw_cv)
                inj1[5].append(dma_xnt)
                inj1[7].append(o1[0])
                inj1[8].append(o1[1])
                inj1[8].append(o1[2])
                inj1[9].append(o1[3])
                inj1[10].append(o2[0])
                inj1[11].append(o2[1])
                inj1[11].append(o2[2])
                inj1[12].append(o2[3])
                inj1[13].append(dma_nw)
                att_loop(0, 0, inj1)
                att_loop(1, 0, edge_injects(0, 0))
                att_loop(0, 1, edge_injects(1, 0))
                g0a, g0b = g_pair(0)
                g1a, g1b = g_pair(1)
                inj4 = edge_injects(0, 1)
                inj4[4].append(dma_xT(0))
                inj4[6].append(g0a)
                inj4[8].append(dma_xT(1))
                inj4[8].append(g0b)
                inj4[10].append(g1a)
                inj4[12].append(dma_xT(2))
                inj4[12].append(g1b)
                inj4[14].append(g_single(0, 2))
                att_loop(1, 1, inj4)

            with loop_cm:
                for _ in range(UNROLL if loop_n > 1 else 1):
                    emit_iter()

            # --- epilogue: finish the last loop4 and redo the output proj --
            tail = edge_injects(1, 1)
            for i in sorted(tail, key=lambda k: (k[0], 0) if isinstance(k, tuple) else (k, 1)):
                for th in tail[i]:
                    th()
            for lhh in range(2):
                for th in out_pass(lhh):
                    th()

    nc.compile()
    return nc


_NC = None


def _get_nc():
    global _NC
    if _NC is None:
        _NC = build_nc()
    return _NC


def make_in_maps(x, Wk, bk, Wq, bq, Wv, bv, Wp, bp):
    import ml_dtypes

    scale = 1.0 / math.sqrt(F)
    in_maps = []
    for c in range(N_CORES):
        b = c // 4
        h0 = 2 * (c % 4)
        hs = [h0, h0 + 1]
        sl = [slice(h * F, (h + 1) * F) for h in hs]
        xb = x[b].astype(np.float32)
        in_maps.append(
            {
                "xT": np.ascontiguousarray(xb.T),
                "xnt": np.ascontiguousarray(
                    xb.reshape(n_blk, F, F).transpose(1, 0, 2).reshape(F, L)
                ),
                "mw": np.ascontiguousarray(
                    np.stack([scale * (Wq[:, s] @ Wk[:, s].T) for s in sl])
                ),
                "nw": np.ascontiguousarray(np.stack([Wv[:, s] @ Wp[s, :] for s in sl])),
                "cv": np.ascontiguousarray(
                    np.stack([scale * (Wk[:, s] @ bq[s]) for s in sl])
                ).reshape(HEADS_PER_CORE, F, 1),
            }
        )
        m = in_maps[-1]
        for k in ("xT", "xnt", "mw", "nw"):
            m[k] = m[k].astype(ml_dtypes.bfloat16)
    return in_maps


def assemble(results, Wp, bv, bp):
    const_row = bv.astype(np.float64) @ Wp.astype(np.float64) + bp
    out = np.empty((B, L, F), np.float32)
    for b in range(B):
        acc = np.zeros((F, L), np.float64)
        for c in range(b * 4, b * 4 + 4):
            acc += results[c]["outT"].astype(np.float64)
        out[b] = (acc.T + const_row[None, :]).astype(np.float32)
    return out


def kernel(x, Wk, bk, Wq, bq, Wv, bv, Wp, bp, _trace=False):
    x = np.asarray(x, np.float32)
    Wk, bk = np.asarray(Wk, np.float32), np.asarray(bk, np.float32)
    Wq, bq = np.asarray(Wq, np.float32), np.asarray(bq, np.float32)
    Wv, bv = np.asarray(Wv, np.float32), np.asarray(bv, np.float32)
    Wp, bp = np.asarray(Wp, np.float32), np.asarray(bp, np.float32)
    nc = _get_nc()
    in_maps = make_in_maps(x, Wk, bk, Wq, bq, Wv, bv, Wp, bp)
    res = run_bass_kernel_spmd(nc, in_maps, list(range(N_CORES)), trace=_trace)
    out = assemble(res.results, Wp, bv, bp)
    if _trace:
        return out, res
    return out


# revision 17
# speedup vs baseline: 1.0678x; 1.0678x over previous
"""Multi-head attention TRN2 kernel.

Problem: x[2,2048,128] -> MHA with 8 heads of dim 128 (inner 1024) -> out[2,2048,128].
Sharding: 8 cores; core c handles batch b=c//4 and heads (2*(c%4), 2*(c%4)+1).
Each core returns the transposed partial output (its two heads' contribution to
y @ Wp); host sums the 4 cores of each batch, transposes, and adds the constant
row bv @ Wp + bp.

Math notes (exact rewrites, not approximations):
- head_dim == n_embd == 128, so the Q/K projections collapse into a single
  128x128 matrix per head: logits = (x M + c) x^T with M = scale*Wq Wk^T and
  c = scale*Wk bq (K bias drops out of softmax entirely; Q bias becomes c).
  The kernel never computes Q or K.
- Likewise V/output projections collapse: out^T = sum_h N_h^T (x^T P_h / r_h)
  with N_h = Wv_h Wp_h, so the kernel never computes V either; the AV matmul
  contracts raw x blocks against the exp tiles, and the V bias contributes
  exactly bv to y (softmax rows sum to 1), folding into the host-side
  constant row.
- Logits have |.| of only a few units, so exp() runs without max-subtraction.

Schedule (v2, exp-bound design):
- The ACT engine is the bottleneck (64 MB of exp per iteration at 1 elem/
  cycle/lane).  QK writes block PAIRS into one fixed [128,2048] PSUM tile and
  exp runs once per pair at N=2048, halving ACT instruction overhead.
- Fully rotated software pipeline: each attention loop defers its last 6 AV
  consumes and its softmax tail (rowsum broadcast, reciprocal, normalize) into
  the NEXT loop's first blocks.  The loop4->loop1 edge crosses the For_i
  boundary, so those deferred emissions reference only fixed-address tiles
  (pt2_fix pair tiles, s3_fix rowsum, ps_z, Zn, G) -- no live Python closures
  over pool tiles cross the edge.  All engines finish each body pass nearly
  simultaneously, so the For_i drain barrier costs little.
- Next-iteration inputs are prefetched inside loop4: each xT chunk DMA is
  emitted right after its last QK read, and the G = (xM+c)^T projection
  chunks follow their xT chunk, so body start has no lead-in work.
- PSUM (8 banks): att pair tile [128,2048] f32 (4) + Z accum [128,1024] f32
  (2) + a 2x[128,512] utility pool (2) time-shared by G projection, rowsum
  broadcast, and output-projection chunks.
- Rowsum: pair-halves add on DVE (s0), mid-tree s1 adds on the otherwise-idle
  Pool engine, tail s1[3]/s2/s3 on DVE; an all-ones stationary matmul
  broadcasts the partition-sums; reciprocal + multiply normalizes.
"""

import sys

sys.path.insert(0, "/opt/trn_rl_repo")

import contextlib
import math
from collections import defaultdict

import numpy as np

import concourse.bass as bass
import concourse.mybir as mybir
import concourse.tile as tile
from concourse import bacc
from concourse.bass_utils import run_bass_kernel_spmd

N_CORES = 8
B, L, F = 2, 2048, 128
NH = 8
HEADS_PER_CORE = 2
LH = 1024  # attention is processed in two l-halves of 1024 columns
F32 = mybir.dt.float32
BF16 = mybir.dt.bfloat16
n_blk = L // F  # 16 a-blocks of 128
n_pair = n_blk // 2  # 8 fused exp pairs per loop
FIRST_CONSUME = 6  # in-loop AV consume j runs at block j+6; j=10..15 deferred


UNROLL = 4  # iterations per For_i body (amortizes the drain barrier)


def build_nc(loop_n: int = 1, sim_init: bool = False):
    del sim_init  # prologue always initializes pipeline-carried state
    assert loop_n == 1 or loop_n % UNROLL == 0, loop_n
    nc = bacc.Bacc("TRN2", target_bir_lowering=False, debug=False, num_devices=N_CORES)
    xT_d = nc.dram_tensor("xT", [F, L], BF16, kind="ExternalInput").ap()
    xnt_d = nc.dram_tensor("xnt", [F, L], BF16, kind="ExternalInput").ap()
    mw_d = nc.dram_tensor("mw", [HEADS_PER_CORE, F, F], BF16, kind="ExternalInput").ap()
    nw_d = nc.dram_tensor("nw", [HEADS_PER_CORE, F, F], BF16, kind="ExternalInput").ap()
    cv_d = nc.dram_tensor("cv", [HEADS_PER_CORE, F, 1], F32, kind="ExternalInput").ap()
    outT_d = nc.dram_tensor("outT", [F, L], BF16, kind="ExternalOutput").ap()

    Exp = mybir.ActivationFunctionType.Exp

    with tile.TileContext(nc) as tc, nc.allow_low_precision(
        reason="bf16 tensors feed the PE at full rate; accumulation stays fp32"
    ):
        with (
            tc.tile_pool(name="fixed", bufs=1) as fixed,
            tc.tile_pool(name="ptp", bufs=8) as ptp,
            tc.tile_pool(name="s0p", bufs=6) as s0p,
            tc.tile_pool(name="s1p", bufs=3) as s1p,
            tc.tile_pool(name="s2p", bufs=2) as s2p,
            tc.tile_pool(name="scr", bufs=2) as scr,
            tc.tile_pool(name="attp", bufs=3, space="PSUM") as attp,
            tc.tile_pool(name="psZp", bufs=1, space="PSUM") as psZp,
        ):
            # ---- fixed-address tiles (live across For_i bodies) ----------
            mw_sb = [
                fixed.tile([F, F], BF16, tag=f"mw{h}", name=f"mw{h}")
                for h in range(HEADS_PER_CORE)
            ]
            nw_sb = [
                fixed.tile([F, F], BF16, tag=f"nw{h}", name=f"nw{h}")
                for h in range(HEADS_PER_CORE)
            ]
            cv_sb = [
                fixed.tile([F, 1], F32, tag=f"cv{h}", name=f"cv{h}")
                for h in range(HEADS_PER_CORE)
            ]
            xTc = [
                fixed.tile([F, 512], BF16, tag=f"xTc{j}", name=f"xTc{j}")
                for j in range(4)
            ]
            xntc = [
                fixed.tile([F, LH], BF16, tag=f"xntc{j}", name=f"xntc{j}")
                for j in range(2)
            ]
            G = [
                fixed.tile([F, L], BF16, tag=f"G{h}", name=f"G{h}")
                for h in range(HEADS_PER_CORE)
            ]
            Zn = [
                fixed.tile([F, L], BF16, tag=f"Zn{h}", name=f"Zn{h}")
                for h in range(HEADS_PER_CORE)
            ]
            # exp tiles for blocks 10..15 are fixed so the deferred AV
            # consumes can be authored without closures over pool tiles
            pt_fix = [
                fixed.tile([F, LH], BF16, tag=f"ptf{k}", name=f"ptf{k}")
                for k in range(6)
            ]
            s3_fix = fixed.tile([F, LH], BF16, tag="s3f", name="s3f")
            s2_fix = fixed.tile([F, LH], BF16, tag="s2f", name="s2f")

            ps_z = psZp.tile([F, LH], F32, tag="zz", name="ps_z")

            def xT_blk(i):  # lhsT for att block i
                return xTc[i // 4][:, (i % 4) * F : (i % 4 + 1) * F]

            def xnt_blk(j):  # lhsT for AV block j
                return xntc[j // 8][:, (j % 8) * F : (j % 8 + 1) * F]

            def g_pair(q):
                """Two thunks writing G[0]/G[1] chunk q via one big psum
                tile's halves (alloc on the first)."""
                box = {}

                def th_h(h):
                    def th():
                        if h == 0:
                            box["t"] = attp.tile([F, LH], F32, tag="big", name=f"g{q}")
                        cs = slice(h * 512, (h + 1) * 512)
                        nc.tensor.matmul(box["t"][:, cs], lhsT=mw_sb[h][:], rhs=xTc[q][:])
                        nc.vector.tensor_scalar_add(
                            G[h][:, q * 512 : (q + 1) * 512], box["t"][:, cs], cv_sb[h][:]
                        )
                    return th

                return th_h(0), th_h(1)

            def g_single(h, q):
                """Standalone G chunk emission (own psum tile); used where
                the paired tile's closure cannot cross the For_i edge."""
                def th():
                    pg = attp.tile([F, LH], F32, tag="big", name=f"gs{h}{q}")
                    nc.tensor.matmul(pg[:, 0:512], lhsT=mw_sb[h][:], rhs=xTc[q][:])
                    nc.vector.tensor_scalar_add(
                        G[h][:, q * 512 : (q + 1) * 512], pg[:, 0:512], cv_sb[h][:]
                    )
                return th

            def out_pass(lhh):
                """alloc+mm(c0), mm(c1)+evac(c0), evac(c1) thunks for the
                output projection of l-half lhh (reads prev-iter Zn)."""
                box = {}
                lo = lhh * LH

                def mm(c):
                    def th():
                        if c == 0:
                            box["t"] = attp.tile([F, LH], F32, tag="big", name=f"o{lhh}")
                        for hh in range(HEADS_PER_CORE):
                            nc.tensor.matmul(
                                box["t"][:, c * 512 : (c + 1) * 512],
                                lhsT=nw_sb[hh][:],
                                rhs=Zn[hh][:, lo + c * 512 : lo + (c + 1) * 512],
                                start=hh == 0,
                                stop=hh == HEADS_PER_CORE - 1,
                            )
                    return th

                def evac(c):
                    def th():
                        ob = scr.tile([F, 512], BF16, tag="ob", name="ob", bufs=4)
                        nc.vector.tensor_copy(ob[:], box["t"][:, c * 512 : (c + 1) * 512])
                        nc.sync.dma_start(
                            out=outT_d[:, lo + c * 512 : lo + (c + 1) * 512], in_=ob[:]
                        )
                    return th

                return mm(0), mm(1), evac(0), evac(1)

            def edge_injects(hp, lp):
                """Deferred tail of the previous loop (head hp, l-half lp):
                6 AV consumes from the fixed exp tiles, rowsum broadcast,
                reciprocals, and the Zn normalize.  Only fixed-address tiles
                are referenced, so this works across the For_i edge."""
                lo = lp * LH
                box = {}

                def consume_prev(j):
                    def th():
                        pf = pt_fix[j - 10]
                        for c in range(2):
                            nc.tensor.matmul(
                                ps_z[:, c * 512 : (c + 1) * 512],
                                lhsT=xnt_blk(j),
                                rhs=pf[:, c * 512 : (c + 1) * 512],
                                start=False,
                                stop=j == n_blk - 1,
                            )
                    return th

                def bcast():
                    pb = attp.tile([F, LH], F32, tag="big", name="bsum")
                    box["b"] = pb
                    for c in range(2):
                        cs = slice(c * 512, (c + 1) * 512)
                        nc.tensor.matmul(pb[:, cs], lhsT=ones_mat[:], rhs=s3_fix[:, cs])

                def recips():
                    for c in range(2):
                        cs = slice(c * 512, (c + 1) * 512)
                        rbc = scr.tile([F, 512], F32, tag="rbc", name="rbc")
                        nc.vector.reciprocal(rbc[:], box["b"][:, cs])
                        box[f"r{c}"] = rbc

                def muls():
                    for c in range(2):
                        nc.vector.tensor_mul(
                            Zn[hp][:, lo + c * 512 : lo + (c + 1) * 512],
                            ps_z[:, c * 512 : (c + 1) * 512],
                            box[f"r{c}"][:],
                        )

                def late_s0():
                    box["s0l"] = s0p.tile([F, LH], BF16, tag="s0", name="s0l")
                    nc.vector.tensor_add(box["s0l"][:], pt_fix[4][:], pt_fix[5][:])

                def late_s3():
                    nc.vector.tensor_add(s3_fix[:], s2_fix[:], box["s0l"][:])

                inj = defaultdict(list)
                inj[0].append(late_s0)
                inj[1].append(late_s3)
                inj[(0, "pre")].append(consume_prev(10))
                inj[(1, "pre")].append(consume_prev(11))
                inj[(2, "pre")].append(consume_prev(12))
                inj[3].append(bcast)
                inj[(3, "pre")].append(consume_prev(13))
                inj[4].append(recips)
                inj[(4, "pre")].append(consume_prev(14))
                inj[(5, "pre")].append(consume_prev(15))
                inj[5].append(muls)
                return inj

            def att_loop(h, lh, inj, post=()):
                """One (head, l-half) attention loop over 16 a-blocks.

                Per block: injected thunks, then the in-loop AV consume
                (j = i-6), then the QK matmuls + exp + rowsum adds.  The
                last 6 consumes and the softmax tail are emitted by the
                NEXT loop via edge_injects."""
                lo = lh * LH
                pts = [None] * n_blk
                s0 = [None] * n_pair
                s1 = [None] * 3
                s2 = [None] * 2
                for i in range(n_blk):
                    for th in inj.get((i, "pre"), ()):
                        th()
                    j = i - FIRST_CONSUME
                    if 0 <= j:
                        for c in range(2):
                            nc.tensor.matmul(
                                ps_z[:, c * 512 : (c + 1) * 512],
                                lhsT=xnt_blk(j),
                                rhs=pts[j][:, c * 512 : (c + 1) * 512],
                                start=j == 0,
                                stop=False,
                            )
                    ps_att = attp.tile([F, LH], F32, tag="big", name="ps_att")
                    for c in range(2):
                        nc.tensor.matmul(
                            ps_att[:, c * 512 : (c + 1) * 512],
                            lhsT=xT_blk(i),
                            rhs=G[h][:, lo + c * 512 : lo + (c + 1) * 512],
                        )
                    for th in inj.get(i, ()):
                        th()
                    pt = (
                        pt_fix[i - 10]
                        if i >= 10
                        else ptp.tile([F, LH], BF16, tag="pt", name="pt")
                    )
                    pts[i] = pt
                    nc.scalar.activation(pt[:], ps_att[:], Exp)
                    # asymmetric rowsum tree: late blocks feed shallow levels
                    # so only ONE add (s3) trails the final exp of the loop.
                    # s1 adds ride the otherwise-idle Pool engine.
                    if i % 2 == 1 and i < 15:
                        p = i // 2
                        s0[p] = s0p.tile([F, LH], BF16, tag="s0", name="s0")
                        nc.vector.tensor_add(s0[p][:], pts[i - 1][:], pts[i][:])
                        if p % 2 == 1 and p <= 5:
                            k = p // 2
                            s1[k] = s1p.tile([F, LH], BF16, tag="s1", name="s1")
                            nc.gpsimd.tensor_add(s1[k][:], s0[2 * k][:], s0[2 * k + 1][:])
                    if i == 11:
                        s2[0] = s2p.tile([F, LH], BF16, tag="s2", name="s2")
                        nc.vector.tensor_add(s2[0][:], s1[0][:], s1[1][:])
                    if i == 14:
                        s2[1] = s2p.tile([F, LH], BF16, tag="s2", name="s2")
                        nc.vector.tensor_add(s2[1][:], s2[0][:], s1[2][:])
                        nc.vector.tensor_add(s2_fix[:], s2[1][:], s0[6][:])
                for th in post:
                    th()

            # --- one-time setup + prologue (outside the timed loop) --------
            ones_stage = fixed.tile([F, F], F32, tag="ones_stage", name="ones_stage")
            nc.vector.memset(ones_stage[:], 1.0)
            ones_mat = fixed.tile([F, F], BF16, tag="ones_mat", name="ones_mat")
            nc.vector.tensor_copy(ones_mat[:], ones_stage[:])
            warm = fixed.tile([F, 1], BF16, tag="warm", name="warm")
            nc.scalar.activation(warm[:], ones_stage[:, 0:1], Exp)
            # initialize pipeline-carried state read by the first body pass
            for h in range(HEADS_PER_CORE):
                nc.vector.memset(Zn[h][:], 0.0)
            for k in range(6):
                nc.vector.memset(pt_fix[k][:], 0.0)
            nc.vector.memset(s3_fix[:], 1.0)
            nc.vector.memset(s2_fix[:], 1.0)
            # open the ps_z accumulation group the first body pass's deferred
            # consumes (start=False) will continue
            for c in range(2):
                nc.tensor.matmul(
                    ps_z[:, c * 512 : (c + 1) * 512],
                    lhsT=ones_mat[:],
                    rhs=pt_fix[0][:, c * 512 : (c + 1) * 512],
                    start=True,
                    stop=False,
                )
            for h in range(HEADS_PER_CORE):
                nc.sync.dma_start(out=mw_sb[h][:], in_=mw_d[h])
            for j in range(4):
                nc.sync.dma_start(out=xTc[j][:], in_=xT_d[:, j * 512 : (j + 1) * 512])
            for h in range(HEADS_PER_CORE):
                nc.sync.dma_start(out=cv_sb[h][:], in_=cv_d[h])
            for h in range(HEADS_PER_CORE):
                nc.sync.dma_start(out=nw_sb[h][:], in_=nw_d[h])
            for j in range(2):
                nc.sync.dma_start(out=xntc[j][:], in_=xnt_d[:, j * LH : (j + 1) * LH])
            for q in range(4):
                a, b = g_pair(q)
                a()
                b()

            def dma_mw_cv():
                for h in range(HEADS_PER_CORE):
                    nc.sync.dma_start(out=mw_sb[h][:], in_=mw_d[h])
                for h in range(HEADS_PER_CORE):
                    nc.sync.dma_start(out=cv_sb[h][:], in_=cv_d[h])

            def dma_nw():
                for h in range(HEADS_PER_CORE):
                    nc.sync.dma_start(out=nw_sb[h][:], in_=nw_d[h])

            def dma_xnt():
                for j in range(2):
                    nc.sync.dma_start(
                        out=xntc[j][:], in_=xnt_d[:, j * LH : (j + 1) * LH]
                    )

            def dma_xT(c):
                def th():
                    nc.sync.dma_start(
                        out=xTc[c][:], in_=xT_d[:, c * 512 : (c + 1) * 512]
                    )
                return th

            loop_cm = (
                tc.For_i(
                    0,
                    loop_n // UNROLL,
                    1,
                    hint_engines=(
                        mybir.EngineType.PE,
                        mybir.EngineType.Activation,
                        mybir.EngineType.DVE,
                        mybir.EngineType.SP,
                        mybir.EngineType.Pool,
                    ),
                )
                if loop_n > 1
                else contextlib.nullcontext()
            )
            def emit_iter():
                # loop1: prev body's loop4 tail + leftover G chunks + dmas +
                # out-projection of the previous iteration's Zn
                g3a, g3b = g_pair(3)
                o1 = out_pass(0)
                o2 = out_pass(1)
                inj1 = edge_injects(1, 1)
                inj1[0].insert(0, g3a)
                inj1[0].insert(0, dma_xT(3))
                inj1[1].insert(0, g3b)
                inj1[2].insert(0, g_single(1, 2))
                inj1[2].append(dma_mw_cv)
                inj1[5].append(dma_xnt)
                inj1[7].append(o1[0])
                inj1[8].append(o1[1])
                inj1[8].append(o1[2])
                inj1[9].append(o1[3])
                inj1[10].append(o2[0])
                inj1[11].append(o2[1])
                inj1[11].append(o2[2])
                inj1[12].append(o2[3])
                inj1[13].append(dma_nw)
                att_loop(0, 0, inj1)
                att_loop(1, 0, edge_injects(0, 0))
                att_loop(0, 1, edge_injects(1, 0))
                g0a, g0b = g_pair(0)
                g1a, g1b = g_pair(1)
                inj4 = edge_injects(0, 1)
                inj4[4].append(dma_xT(0))
                inj4[6].append(g0a)
                inj4[8].append(dma_xT(1))
                inj4[8].append(g0b)
                inj4[10].append(g1a)
                inj4[12].append(dma_xT(2))
                inj4[12].append(g1b)
                inj4[14].append(g_single(0, 2))
                att_loop(1, 1, inj4)

            with loop_cm:
                for _ in range(UNROLL if loop_n > 1 else 1):
                    emit_iter()

            # --- epilogue: finish the last loop4 and redo the output proj --
            tail = edge_injects(1, 1)
            for i in sorted(tail, key=lambda k: (k[0], 0) if isinstance(k, tuple) else (k, 1)):
                for th in tail[i]:
                    th()
            for lhh in range(2):
                for th in out_pass(lhh):
                    th()

    nc.compile()
    return nc


_NC = None


def _get_nc():
    global _NC
    if _NC is None:
        _NC = build_nc()
    return _NC


def make_in_maps(x, Wk, bk, Wq, bq, Wv, bv, Wp, bp):
    import ml_dtypes

    scale = 1.0 / math.sqrt(F)
    in_maps = []
    for c in range(N_CORES):
        b = c // 4
        h0 = 2 * (c % 4)
        hs = [h0, h0 + 1]
        sl = [slice(h * F, (h + 1) * F) for h in hs]
        xb = x[b].astype(np.float32)
        in_maps.append(
            {
                "xT": np.ascontiguousarray(xb.T),
                "xnt": np.ascontiguousarray(
                    xb.reshape(n_blk, F, F).transpose(1, 0, 2).reshape(F, L)
                ),
                "mw": np.ascontiguousarray(
                    np.stack([scale * (Wq[:, s] @ Wk[:, s].T) for s in sl])
                ),
                "nw": np.ascontiguousarray(np.stack([Wv[:, s] @ Wp[s, :] for s in sl])),
                "cv": np.ascontiguousarray(
                    np.stack([scale * (Wk[:, s] @ bq[s]) for s in sl])
                ).reshape(HEADS_PER_CORE, F, 1),
            }
        )
        m = in_maps[-1]
        for k in ("xT", "xnt", "mw", "nw"):
            m[k] = m[k].astype(ml_dtypes.bfloat16)
    return in_maps


def assemble(results, Wp, bv, bp):
    const_row = bv.astype(np.float64) @ Wp.astype(np.float64) + bp
    out = np.empty((B, L, F), np.float32)
    for b in range(B):
        acc = np.zeros((F, L), np.float64)
        for c in range(b * 4, b * 4 + 4):
            acc += results[c]["outT"].astype(np.float64)
        out[b] = (acc.T + const_row[None, :]).astype(np.float32)
    return out


def kernel(x, Wk, bk, Wq, bq, Wv, bv, Wp, bp, _trace=False):
    x = np.asarray(x, np.float32)
    Wk, bk = np.asarray(Wk, np.float32), np.asarray(bk, np.float32)
    Wq, bq = np.asarray(Wq, np.float32), np.asarray(bq, np.float32)
    Wv, bv = np.asarray(Wv, np.float32), np.asarray(bv, np.float32)
    Wp, bp = np.asarray(Wp, np.float32), np.asarray(bp, np.float32)
    nc = _get_nc()
    in_maps = make_in_maps(x, Wk, bk, Wq, bq, Wv, bv, Wp, bp)
    res = run_bass_kernel_spmd(nc, in_maps, list(range(N_CORES)), trace=_trace)
    out = assemble(res.results, Wp, bv, bp)
    if _trace:
        return out, res
    return out
